# revision 9
# baseline (speedup 1.0000x reference)
"""Trainium2 Bass kernel for nn_CombineConcat (pairwise broadcast+concat).

reference semantics (per batch b):
  out[b, i*N + j, 0:D]   = x1[b, i, :]
  out[b, i*N + j, D:2*D] = x2[b, j, :]

Shapes (hardcoded): x1, x2 = [16, 128, 256] f32 -> out = [16, 16384, 512] f32.

Strategy: data-parallel over the batch dim, 2 batches per core on 8 cores.
Write-bandwidth bound: each core writes 64 MB (reads 512 KB). The 16 SDMA
engines sustain ~380-408 GB/s with 2KB-per-partition descriptors across the
two HWDGE queues, so the data floor is ~170 us; everything else must hide
behind the write stream.

Structure (v4):
 - 64-slot SBUF ring; slot k = [x1_i | x2] row block for block i (k=i%64).
   One 3-dim DMA descriptor per group of up to 8 blocks (2 MB) keeps
   descriptor-issue cost trivial.
 - x1 rows are broadcast into the ring by gpsimd partition_broadcast from
   small partition-0 stages (BIR requires pbcast sources at partition 0),
   one 8-row stage load per group, prefetched 4 groups ahead through the
   same HWDGE queues (measured: a separate SWDGE queue slows every HWDGE
   packet by ~12%, so everything stays on the two HWDGE queues).
 - All cross-batch-critical loads are issued up front while the queues are
   empty: both t2 tiles and batch-1's first four stages (dedicated tiles),
   so nothing at the batch boundary waits behind 2MB chunks in a FIFO.
 - The first pbcast after the preamble costs ~4-10us (Q7 cold start): a
   dummy warmup pbcast pays it concurrently with the input loads, and
   batch 0's first 16 blocks bypass pbcast entirely via stride-0
   broadcast-read DMA descriptors (DRAM row read 128x), which also gives
   the SDMA engines useful work during the ramp.
"""

import numpy as np

_B, _N, _D = 16, 128, 256
_NCORES = 8
_BPC = _B // _NCORES  # batches per core

_NC_CACHE = {}


def _build_nc(bpc=_BPC, n=_N, d=_D, k_ring=64, stage_bufs=4, n_dbc=16, n_ded=4):
    import concourse.bacc as bacc
    import concourse.bass as bass
    import concourse.mybir as mybir
    from concourse.tile import TileContext

    f32 = mybir.dt.float32
    nc = bacc.Bacc("TRN2", target_bir_lowering=False, enable_partition_id=False)
    x1 = nc.dram_tensor("x1", [bpc, n, d], f32, kind="ExternalInput")
    x2 = nc.dram_tensor("x2", [bpc, n, d], f32, kind="ExternalInput")
    out = nc.dram_tensor("out", [bpc, n * n, 2 * d], f32, kind="ExternalOutput")
    W = 2 * d  # ring slot width in elements
    G = 8  # blocks per pbcast group / output descriptor
    DG = 4  # blocks per DMA-broadcast group (batch-0 ramp)

    # batch 0: first n_dbc blocks via DMA broadcast-read (no pbcast dep),
    # rest via pbcast groups of G. batch 1+: all pbcast groups.
    scheds = [
        [("dbc", i0, DG) for i0 in range(0, n_dbc, DG)]
        + [("pb", i0, G) for i0 in range(n_dbc, n, G)]
    ] + [[("pb", i0, G) for i0 in range(0, n, G)] for _ in range(bpc - 1)]
    x1f = [x1[b].rearrange("n d -> (n d)") for b in range(bpc)]
    # flat list of pbcast groups in execution order; the first n_ded of
    # batch>=1 get dedicated preloaded stages, the rest rotate via spool.
    pblist = [
        (b, gi) for b in range(bpc) for gi, e in enumerate(scheds[b]) if e[0] == "pb"
    ]
    dedicated = {(b, gi) for b, gi in pblist if b > 0 and scheds[b][gi][1] < n_ded * G}
    slist = [e for e in pblist if e not in dedicated]

    # Separate strict-alternation toggles: chunks must alternate queues
    # exactly (both queues stay deep -> engines stream at ~400 GB/s);
    # mixing small loads into the same toggle bunches chunks on one queue
    # and the uneven mix degrades per-packet efficiency ~12%.
    qtog = {"chunk": 0, "load": 0}

    with TileContext(nc) as tc:

        def alt(kind="load"):
            qtog[kind] ^= 1
            return nc.sync if qtog[kind] else nc.scalar

        with (
            tc.tile_pool(name="io", bufs=1) as iop,
            tc.tile_pool(name="x1s", bufs=stage_bufs) as spool,
            tc.tile_pool(name="ring", bufs=1) as rp,
        ):
            t2s = [
                iop.tile([n, d], f32, tag=f"t2_{b}", name=f"t2_{b}")
                for b in range(bpc)
            ]
            wsrc = iop.tile([1, 8], f32, tag="wsrc", name="wsrc")
            wdst = iop.tile([n, 8], f32, tag="wdst", name="wdst")

            # ring: slot k holds [x1_i | x2] for block i (k = i % k_ring)
            RB = rp.tile([n, k_ring * W], f32, tag="RB")
            RBv = RB[:].rearrange("p (k h c) -> p k h c", k=k_ring, c=d)

            def stage_load(b, gi):
                """DMA x1[b] rows [i0, i0+G) flat onto a partition-0 stage."""
                _, i0, Gx = scheds[b][gi]
                if (b, gi) in dedicated:
                    s = iop.tile(
                        [1, G * d], f32, tag=f"x1d_{b}_{gi}", name=f"x1d_{b}_{gi}"
                    )
                else:
                    s = spool.tile([1, G * d], f32, tag="x1s", name=f"x1s_{b}_{gi}")
                alt().dma_start(
                    out=s[0:1, 0 : Gx * d], in_=x1f[b][i0 * d : (i0 + Gx) * d]
                )
                return s

            # ---- prologue: all loads the ramp & boundary depend on ----
            nc.scalar.dma_start(out=t2s[0][:], in_=x2[0])
            for b in range(1, bpc):
                nc.sync.dma_start(out=t2s[b][:], in_=x2[b])
            # x1 DMA-broadcast descriptors for batch 0's ramp blocks: src
            # reads row i0+g 128 times (partition-dim stride 0).
            for i0 in range(0, n_dbc, DG):
                base = x1f[0][i0 * d : (i0 + DG) * d]
                src = bass.AP(
                    tensor=base.tensor,
                    offset=base.offset,
                    ap=[[0, n], [d, DG], [1, d]],
                )
                alt("load").dma_start(out=RBv[:, i0 : i0 + DG, 0, :], in_=src)
            # warmup: pay the Q7 pbcast cold-start before the real ones.
            nc.gpsimd.memset(wsrc[0:1, :], 0)
            nc.gpsimd.partition_broadcast(wdst[:], wsrc[0:1, :], opt=False)
            # stage preloads: batch 0's first `stage_bufs` groups plus the
            # dedicated boundary stages, all while the queues are empty.
            stages = {}
            for e in slist[:stage_bufs]:
                stages[e] = stage_load(*e)
            for e in sorted(dedicated):
                stages[e] = stage_load(*e)

            # ---- main loop ----
            spfx = stage_bufs  # next index into slist to prefetch
            for b in range(bpc):
                ob = out[b]  # [n*n, 2d]
                for gi, (kind, i0, Gx) in enumerate(scheds[b]):
                    k0 = i0 % k_ring
                    # x2 halves: written once per batch per slot (slots are
                    # reused within a batch with identical x2 content).
                    if i0 < k_ring:
                        for k in range(k0, k0 + Gx):
                            nc.vector.tensor_copy(out=RBv[:, k, 1, :], in_=t2s[b][:])
                    if kind == "pb":
                        # x1 halves: broadcast rows i0..i0+Gx-1 from stage.
                        nc.gpsimd.partition_broadcast(
                            RBv[:, k0 : k0 + Gx, 0, :],
                            stages.pop((b, gi))[0:1, 0 : Gx * d],
                            opt=False,
                        )
                    # one descriptor for the whole group: src [128, Gx*512],
                    # dst rows i0*128..(i0+Gx)*128 iterated (p, g, c).
                    alt("chunk").dma_start(
                        out=ob[i0 * n : (i0 + Gx) * n, :].rearrange(
                            "(g p) c -> p g c", g=Gx
                        ),
                        in_=RB[:, k0 * W : (k0 + Gx) * W],
                    )
                    # prefetch the next spool stage a few groups ahead
                    if kind == "pb" and (b, gi) not in dedicated:
                        if spfx < len(slist):
                            stages[slist[spfx]] = stage_load(*slist[spfx])
                            spfx += 1
    nc.finalize()
    return nc


def _get_nc():
    if "nc" not in _NC_CACHE:
        _NC_CACHE["nc"] = _build_nc()
    return _NC_CACHE["nc"]


def _run(x1, x2, trace=False):
    """Run the kernel on 8 cores; returns (output, BassKernelResults)."""
    from concourse.bass_utils import run_bass_kernel_spmd

    nc = _get_nc()
    x1 = np.ascontiguousarray(np.asarray(x1, dtype=np.float32))
    x2 = np.ascontiguousarray(np.asarray(x2, dtype=np.float32))
    in_maps = [
        {
            "x1": x1[c * _BPC : (c + 1) * _BPC],
            "x2": x2[c * _BPC : (c + 1) * _BPC],
        }
        for c in range(_NCORES)
    ]
    res = run_bass_kernel_spmd(
        nc, in_maps, core_ids=list(range(_NCORES)), trace=trace
    )
    out = np.concatenate([r["out"] for r in res.results], axis=0)
    return out, res


def kernel(x1, x2):
    out, _ = _run(x1, x2, trace=False)
    return out


# revision 11
# speedup vs baseline: 1.1312x; 1.1312x over previous
"""Trainium2 Bass kernel for nn_CombineConcat (pairwise broadcast+concat).

reference semantics (per batch b):
  out[b, i*N + j, 0:D]   = x1[b, i, :]
  out[b, i*N + j, D:2*D] = x2[b, j, :]

Shapes (hardcoded): x1, x2 = [16, 128, 256] f32 -> out = [16, 16384, 512] f32.

Strategy: data-parallel over the batch dim, 2 batches per core on 8 cores.
Write-bandwidth bound: each core writes 64 MB (reads 512 KB). The 16 SDMA
engines sustain ~380-408 GB/s with 2KB-per-partition descriptors across the
two HWDGE queues, so the data floor is ~170 us; everything else must hide
behind the write stream.

Structure (v4):
 - 64-slot SBUF ring; slot k = [x1_i | x2] row block for block i (k=i%64).
   One 3-dim DMA descriptor per group of up to 8 blocks (2 MB) keeps
   descriptor-issue cost trivial.
 - x1 rows are broadcast into the ring by gpsimd partition_broadcast from
   small partition-0 stages (BIR requires pbcast sources at partition 0),
   one 8-row stage load per group, prefetched 4 groups ahead through the
   same HWDGE queues (measured: a separate SWDGE queue slows every HWDGE
   packet by ~12%, so everything stays on the two HWDGE queues).
 - All cross-batch-critical loads are issued up front while the queues are
   empty: both t2 tiles and batch-1's first four stages (dedicated tiles),
   so nothing at the batch boundary waits behind 2MB chunks in a FIFO.
 - The first pbcast after the preamble costs ~4-10us (Q7 cold start): a
   dummy warmup pbcast pays it concurrently with the input loads, and
   batch 0's first 16 blocks bypass pbcast entirely via stride-0
   broadcast-read DMA descriptors (DRAM row read 128x), which also gives
   the SDMA engines useful work during the ramp.
"""

import numpy as np

_B, _N, _D = 16, 128, 256
_NCORES = 8
_BPC = _B // _NCORES  # batches per core

_NC_CACHE = {}


def _build_nc(bpc=_BPC, n=_N, d=_D, k_ring=64, stage_bufs=4, n_dbc=16, n_ded=4):
    import concourse.bacc as bacc
    import concourse.bass as bass
    import concourse.mybir as mybir
    from concourse.tile import TileContext

    f32 = mybir.dt.float32
    nc = bacc.Bacc("TRN2", target_bir_lowering=False, enable_partition_id=False)
    x1 = nc.dram_tensor("x1", [bpc, n, d], f32, kind="ExternalInput")
    x2 = nc.dram_tensor("x2", [bpc, n, d], f32, kind="ExternalInput")
    out = nc.dram_tensor("out", [bpc, n * n, 2 * d], f32, kind="ExternalOutput")
    W = 2 * d  # ring slot width in elements
    G = 8  # blocks per pbcast group / output descriptor
    DG = 4  # blocks per DMA-broadcast group (batch-0 ramp)

    # batch 0: first n_dbc blocks via DMA broadcast-read (no pbcast dep),
    # rest via pbcast groups of G. batch 1+: all pbcast groups.
    scheds = [
        [("dbc", i0, DG) for i0 in range(0, n_dbc, DG)]
        + [("pb", i0, G) for i0 in range(n_dbc, n, G)]
    ] + [[("pb", i0, G) for i0 in range(0, n, G)] for _ in range(bpc - 1)]
    x1f = [x1[b].rearrange("n d -> (n d)") for b in range(bpc)]
    # flat list of pbcast groups in execution order; the first n_ded of
    # batch>=1 get dedicated preloaded stages, the rest rotate via spool.
    pblist = [
        (b, gi) for b in range(bpc) for gi, e in enumerate(scheds[b]) if e[0] == "pb"
    ]
    dedicated = {(b, gi) for b, gi in pblist if b > 0 and scheds[b][gi][1] < n_ded * G}
    slist = [e for e in pblist if e not in dedicated]

    # Queue assignment: ALL output chunks on one queue, ALL small loads on
    # the other. One deep queue streams at ~380 GB/s; splitting 2MB chunks
    # across both queues makes the engines round-robin between two bursty
    # heads and costs ~6-12% per packet (measured), as does any window
    # where chunks leak onto the load queue.
    with TileContext(nc) as tc:

        def alt(kind="load"):
            return nc.sync if kind == "chunk" else nc.scalar

        with (
            tc.tile_pool(name="io", bufs=1) as iop,
            tc.tile_pool(name="x1s", bufs=stage_bufs) as spool,
            tc.tile_pool(name="ring", bufs=1) as rp,
        ):
            t2s = [
                iop.tile([n, d], f32, tag=f"t2_{b}", name=f"t2_{b}")
                for b in range(bpc)
            ]
            wsrc = iop.tile([1, 8], f32, tag="wsrc", name="wsrc")
            wdst = iop.tile([n, 8], f32, tag="wdst", name="wdst")

            # ring: slot k holds [x1_i | x2] for block i (k = i % k_ring)
            RB = rp.tile([n, k_ring * W], f32, tag="RB")
            RBv = RB[:].rearrange("p (k h c) -> p k h c", k=k_ring, c=d)

            def stage_load(b, gi):
                """DMA x1[b] rows [i0, i0+G) flat onto a partition-0 stage."""
                _, i0, Gx = scheds[b][gi]
                if (b, gi) in dedicated:
                    s = iop.tile(
                        [1, G * d], f32, tag=f"x1d_{b}_{gi}", name=f"x1d_{b}_{gi}"
                    )
                else:
                    s = spool.tile([1, G * d], f32, tag="x1s", name=f"x1s_{b}_{gi}")
                alt().dma_start(
                    out=s[0:1, 0 : Gx * d], in_=x1f[b][i0 * d : (i0 + Gx) * d]
                )
                return s

            # ---- prologue: all loads the ramp & boundary depend on ----
            for b in range(bpc):
                nc.scalar.dma_start(out=t2s[b][:], in_=x2[b])
            # x1 DMA-broadcast descriptors for batch 0's ramp blocks: src
            # reads row i0+g 128 times (partition-dim stride 0).
            for i0 in range(0, n_dbc, DG):
                base = x1f[0][i0 * d : (i0 + DG) * d]
                src = bass.AP(
                    tensor=base.tensor,
                    offset=base.offset,
                    ap=[[0, n], [d, DG], [1, d]],
                )
                alt("load").dma_start(out=RBv[:, i0 : i0 + DG, 0, :], in_=src)
            # warmup: pay the Q7 pbcast cold-start before the real ones.
            nc.gpsimd.memset(wsrc[0:1, :], 0)
            nc.gpsimd.partition_broadcast(wdst[:], wsrc[0:1, :], opt=False)
            # stage preloads: batch 0's first `stage_bufs` groups plus the
            # dedicated boundary stages, all while the queues are empty.
            stages = {}
            for e in slist[:stage_bufs]:
                stages[e] = stage_load(*e)
            for e in sorted(dedicated):
                stages[e] = stage_load(*e)

            # ---- main loop ----
            spfx = stage_bufs  # next index into slist to prefetch
            for b in range(bpc):
                ob = out[b]  # [n*n, 2d]
                for gi, (kind, i0, Gx) in enumerate(scheds[b]):
                    k0 = i0 % k_ring
                    # x2 halves: written once per batch per slot (slots are
                    # reused within a batch with identical x2 content).
                    if i0 < k_ring:
                        for k in range(k0, k0 + Gx):
                            nc.vector.tensor_copy(out=RBv[:, k, 1, :], in_=t2s[b][:])
                    if kind == "pb":
                        # x1 halves: broadcast rows i0..i0+Gx-1 from stage.
                        nc.gpsimd.partition_broadcast(
                            RBv[:, k0 : k0 + Gx, 0, :],
                            stages.pop((b, gi))[0:1, 0 : Gx * d],
                            opt=False,
                        )
                    # one descriptor for the whole group: src [128, Gx*512],
                    # dst rows i0*128..(i0+Gx)*128 iterated (p, g, c).
                    alt("chunk").dma_start(
                        out=ob[i0 * n : (i0 + Gx) * n, :].rearrange(
                            "(g p) c -> p g c", g=Gx
                        ),
                        in_=RB[:, k0 * W : (k0 + Gx) * W],
                    )
                    # prefetch the next spool stage a few groups ahead
                    if kind == "pb" and (b, gi) not in dedicated:
                        if spfx < len(slist):
                            stages[slist[spfx]] = stage_load(*slist[spfx])
                            spfx += 1
    nc.finalize()
    return nc


def _get_nc():
    if "nc" not in _NC_CACHE:
        _NC_CACHE["nc"] = _build_nc()
    return _NC_CACHE["nc"]


def _run(x1, x2, trace=False):
    """Run the kernel on 8 cores; returns (output, BassKernelResults)."""
    from concourse.bass_utils import run_bass_kernel_spmd

    nc = _get_nc()
    x1 = np.ascontiguousarray(np.asarray(x1, dtype=np.float32))
    x2 = np.ascontiguousarray(np.asarray(x2, dtype=np.float32))
    in_maps = [
        {
            "x1": x1[c * _BPC : (c + 1) * _BPC],
            "x2": x2[c * _BPC : (c + 1) * _BPC],
        }
        for c in range(_NCORES)
    ]
    res = run_bass_kernel_spmd(
        nc, in_maps, core_ids=list(range(_NCORES)), trace=trace
    )
    out = np.concatenate([r["out"] for r in res.results], axis=0)
    return out, res


def kernel(x1, x2):
    out, _ = _run(x1, x2, trace=False)
    return out
